# revision 9
# baseline (speedup 1.0000x reference)
"""BiLSTM language-model kernel for 8 Trainium2 NeuronCores.

Reference computation (backward LSTM direction is dead code in the reference):
    x  = emb[input]                          # (B=8, T=512, E=512)
    xg = x @ W_ih_fwd.T + b_ih + b_hh        # (T, B, 4H)
    h  = LSTM-scan(xg, W_hh_fwd)             # (T, B, H)
    out = h @ W_out.T + b_out                # (B, T, V=32000)

Distribution strategy:
  - Embedding lookup: host-side (pure indexed copy of inputs).
  - xg GEMM: sharded over T across the 8 cores (fp32r), AllGathered.
  - LSTM scan: replicated on all 8 cores (sequential recurrence; weights
    stationary in bf16 to exploit fast-weight-load; gates kept with the
    gate dim on partitions so elementwise work is ~free).
  - Output GEMM: vocab-sharded (4000 vocab rows per core, fp32r), each core
    writes its (8, 512, 4000) slice; host concatenates.
"""

import os
import numpy as np
import ml_dtypes

import concourse.bass as bass
import concourse.tile as tile
from concourse import bacc, mybir
from concourse.bass_utils import run_bass_kernel_spmd

_T_BUILD = int(os.environ.get("BILSTM_T_BUILD", "512"))  # dev knob: scan length


def _wire_ntff_hook():
    """The agent image's antenv lacks axon_hooks; synthesize it so
    run_bass_kernel_spmd(trace=True) can capture NTFF profiles."""
    import sys
    import types
    try:
        from antenv.axon_hooks import get_axon_ntff_profile_hook  # noqa: F401
        return  # already present
    except ImportError:
        pass
    try:
        import antenv
        from trn_agent_boot.trn_boot import _ntff_profile_via_ctypes
        mod = types.ModuleType("antenv.axon_hooks")
        _store = [None]
        mod.set_axon_ntff_profile_hook = lambda h: _store.__setitem__(0, h)
        mod.get_axon_ntff_profile_hook = lambda: _store[0]
        sys.modules["antenv.axon_hooks"] = mod
        antenv.axon_hooks = mod
        mod.set_axon_ntff_profile_hook(
            _ntff_profile_via_ctypes("/opt/axon/libaxon_pjrt.so"))
    except Exception:
        pass


_wire_ntff_hook()

F32 = mybir.dt.float32
F32R = mybir.dt.float32r
BF16 = mybir.dt.bfloat16
AF = mybir.ActivationFunctionType

N_CORES = 8
B, T, E, H, V = 8, 512, 512, 512, 32000
G = 4 * H                   # 2048 gate rows
NM = G // 128               # 16 gate M-tiles
NK = H // 128               # 4 contraction K-tiles
TC = T // N_CORES           # 64 timesteps per core for the xg GEMM
NS = 4                      # xg AllGather sub-chunks per core (16 steps each)
VC = V // N_CORES           # 4000 vocab rows per core
VCH = 8                     # vocab chunks in output GEMM
VN = VC // VCH              # 500 vocab per chunk
NBT = (B * T) // 128        # 32 bt-tiles in the output GEMM

# gate m-tile order: i(0:4) f(4:8) o(8:12) g(12:16)  [o before g so that the
# three sigmoid gates are contiguous on the free axis]
_PERM = np.concatenate([np.arange(0, H), np.arange(H, 2 * H),
                        np.arange(3 * H, 4 * H), np.arange(2 * H, 3 * H)])

_CACHE = {}


def _build():
    if "nc" in _CACHE:
        return _CACHE["nc"]
    nc = bacc.Bacc("TRN2", target_bir_lowering=False, debug=False,
                   num_devices=N_CORES)

    # ---- DRAM I/O ----
    xt_dram = nc.dram_tensor("xt", [E, TC * B], F32R, kind="ExternalInput")
    wih_dram = nc.dram_tensor("wih", [E, G], F32R, kind="ExternalInput")
    whh_dram = nc.dram_tensor("whh", [H, G], BF16, kind="ExternalInput")
    bg_dram = nc.dram_tensor("bg", [128, NM], F32, kind="ExternalInput")
    wout_dram = nc.dram_tensor("wout", [H, VC], F32R, kind="ExternalInput")
    bout_dram = nc.dram_tensor("bout", [128, VC], F32, kind="ExternalInput")
    out_dram = nc.dram_tensor("out", [B, T, VC], F32, kind="ExternalOutput")

    xg_mine = [nc.dram_tensor(f"xg_mine{s}", [NM, 128, 16 * B], F32)
               for s in range(NS)]
    xg_all = [nc.dram_tensor(f"xg_all{s}", [N_CORES, NM, 128, 16 * B], F32,
                             addr_space="Shared") for s in range(NS)]

    with tile.TileContext(nc) as tc:
        with (
            tc.tile_pool(name="wbig", bufs=1) as wbig,      # W_ihT then W_outT
            tc.tile_pool(name="wsmall", bufs=1) as wsmall,  # persistent weights
            tc.tile_pool(name="state", bufs=1) as statep,   # scan state
            tc.tile_pool(name="hs", bufs=NBT) as hsp,       # scan outputs
            tc.tile_pool(name="xgst", bufs=3) as xgst,      # xg staging
            tc.tile_pool(name="xgpre", bufs=4) as xgpre,    # scan xg prefetch
            tc.tile_pool(name="ovec", bufs=4) as ovec,      # out staging
            tc.tile_pool(name="ps", bufs=4, space="PSUM") as psp,
        ):
            # ================= phase 0: weight loads =================
            wih = wbig.tile([128, NK, G], F32R, tag="wbig")
            nc.sync.dma_start(wih[:], wih_dram[:].rearrange("(k p) g -> p k g", p=128))
            whh = wsmall.tile([128, NK, G], BF16)
            nc.sync.dma_start(whh[:], whh_dram[:].rearrange("(k p) g -> p k g", p=128))
            bg = wsmall.tile([128, NM], F32)
            nc.sync.dma_start(bg[:], bg_dram[:])
            xt = wsmall.tile([128, NK, TC * B], F32R)
            nc.sync.dma_start(xt[:], xt_dram[:].rearrange("(k p) n -> p k n", p=128))

            # ================= phase 1: xg GEMM (my T-chunk) =================
            for m in range(NM):
                ps = psp.tile([128, TC * B], F32, tag="ps")
                for k in range(NK):
                    nc.tensor.matmul(
                        ps[:], wih[:, k, 128 * m:128 * (m + 1)], xt[:, k, :],
                        start=(k == 0), stop=(k == NK - 1))
                st = xgst.tile([128, TC * B], F32)
                nc.scalar.activation(st[:], ps[:], AF.Identity,
                                     bias=bg[:, m:m + 1])
                for s in range(NS):
                    nc.sync.dma_start(xg_mine[s][m], st[:, 128 * s:128 * (s + 1)])

            # ================= phase 2: AllGather xg =================
            for s in range(NS):
                nc.gpsimd.collective_compute(
                    "AllGather", mybir.AluOpType.bypass,
                    ins=[xg_mine[s][:]], outs=[xg_all[s][:]],
                    replica_groups=[list(range(N_CORES))])

            # W_outT load (overlaps the scan; reuses the W_ihT slot)
            wout = wbig.tile([128, NK, VC], F32R, tag="wbig")
            nc.sync.dma_start(wout[:], wout_dram[:].rearrange("(k p) v -> p k v", p=128))
            bout = wsmall.tile([128, VC], F32)
            nc.sync.dma_start(bout[:], bout_dram[:])

            # ================= phase 3: LSTM scan =================
            c_t = statep.tile([128, NK, B], F32)
            hbf = statep.tile([128, NK, B], BF16)
            t1 = statep.tile([128, NK, B], F32)
            t2 = statep.tile([128, NK, B], F32)
            tnc = statep.tile([128, NK, B], F32)
            gsb = statep.tile([128, NM, B], F32)
            nc.vector.memset(c_t[:], 0.0)
            nc.vector.memset(hbf[:], 0.0)

            hs = [hsp.tile([128, NK, 128], F32R, tag="hs", name=f"hs{j}")
                  for j in range(NBT)]
            for hst in hs:
                nc.vector.memset(hst[:].bitcast(F32), 0.0)

            for t in range(_T_BUILD):
                cc, ss, tl = t // TC, (t % TC) // 16, t % 16
                xg_t = xgpre.tile([128, NM, B], F32)
                nc.sync.dma_start(
                    xg_t[:],
                    xg_all[ss][cc, :, :, B * tl:B * (tl + 1)].rearrange(
                        "m p b -> p m b"))

                ps = psp.tile([128, NM, B], F32, tag="ps")
                for m in range(NM):
                    for k in range(NK):
                        nc.tensor.matmul(
                            ps[:, m, :],
                            whh[:, k, 128 * m:128 * (m + 1)], hbf[:, k, :],
                            start=(k == 0), stop=(k == NK - 1))

                # gates = hg + xg ; nonlinearities
                nc.vector.tensor_add(gsb[:], ps[:], xg_t[:])
                nc.scalar.activation(gsb[:, 0:12, :], gsb[:, 0:12, :], AF.Sigmoid)
                nc.scalar.activation(gsb[:, 12:16, :], gsb[:, 12:16, :], AF.Tanh)
                # c = f*c + i*g ; h = o*tanh(c)
                nc.vector.tensor_mul(t1[:], gsb[:, 0:4, :], gsb[:, 12:16, :])
                nc.vector.tensor_mul(t2[:], gsb[:, 4:8, :], c_t[:])
                nc.vector.tensor_add(c_t[:], t1[:], t2[:])
                nc.scalar.activation(tnc[:], c_t[:], AF.Tanh)
                j, o = t // 16, t % 16
                h_slice = hs[j][:, :, B * o:B * (o + 1)]
                nc.vector.tensor_mul(h_slice, gsb[:, 8:12, :], tnc[:])
                nc.scalar.activation(hbf[:], h_slice, AF.Copy)

            # ================= phase 4: output GEMM =================
            for j in range(NBT):
                for v in range(VCH):
                    ps = psp.tile([128, VN], F32, tag="ps")
                    for k in range(NK):
                        nc.tensor.matmul(
                            ps[:], hs[j][:, k, :], wout[:, k, VN * v:VN * (v + 1)],
                            start=(k == 0), stop=(k == NK - 1))
                    ot = ovec.tile([128, VN], F32)
                    nc.vector.tensor_add(ot[:], ps[:], bout[:, VN * v:VN * (v + 1)])
                    # partition p = 8*t_local + b ; bt-tile j covers t in [16j, 16j+16)
                    dst = out_dram[:, 16 * j:16 * (j + 1), VN * v:VN * (v + 1)]
                    nc.sync.dma_start(dst.rearrange("b t v -> t b v"), ot[:])

    nc.compile()
    _CACHE["nc"] = nc
    return nc


def kernel(**inputs) -> np.ndarray:
    inp = np.asarray(inputs["input"])
    emb = np.asarray(inputs["emb"], dtype=np.float32)
    W_ih = np.asarray(inputs["W_ih_fwd"], dtype=np.float32)
    b_ih = np.asarray(inputs["b_ih_fwd"], dtype=np.float32)
    W_hh = np.asarray(inputs["W_hh_fwd"], dtype=np.float32)
    b_hh = np.asarray(inputs["b_hh_fwd"], dtype=np.float32)
    W_out = np.asarray(inputs["W_out"], dtype=np.float32)
    b_out = np.asarray(inputs["b_out"], dtype=np.float32)

    nc = _build()

    # host-side input prep
    x = emb[inp]                                   # (B, T, E)
    wihT = np.ascontiguousarray(W_ih[_PERM].T)     # (E, G) permuted gate order
    whhT = np.ascontiguousarray(W_hh[_PERM].T).astype(ml_dtypes.bfloat16)
    bgv = (b_ih + b_hh)[_PERM].reshape(NM, 128).T.copy()  # (128, NM)

    in_maps = []
    for c in range(N_CORES):
        xc = x[:, TC * c:TC * (c + 1), :]          # (B, TC, E)
        xt = np.ascontiguousarray(xc.transpose(2, 1, 0).reshape(E, TC * B))
        wo = np.ascontiguousarray(W_out[VC * c:VC * (c + 1)].T)  # (H, VC)
        bo = np.tile(b_out[VC * c:VC * (c + 1)][None, :], (128, 1))
        in_maps.append({
            "xt": xt, "wih": wihT, "whh": whhT, "bg": bgv,
            "wout": wo, "bout": np.ascontiguousarray(bo),
        })

    res = run_bass_kernel_spmd(
        nc, in_maps, core_ids=list(range(N_CORES)),
        trace=bool(int(os.environ.get("BILSTM_TRACE", "0"))))
    _CACHE["last_res"] = res
    out = np.concatenate([res.results[c]["out"] for c in range(N_CORES)], axis=2)
    return out.astype(np.float32)


if __name__ == "__main__":
    rng = np.random.default_rng(0)
    pass


# revision 15
# speedup vs baseline: 1.2841x; 1.2841x over previous
"""BiLSTM language-model kernel for 8 Trainium2 NeuronCores.

Reference computation (backward LSTM direction is dead code in the reference):
    x  = emb[input]                          # (B=8, T=512, E=512)
    xg = x @ W_ih_fwd.T + b_ih + b_hh        # (T, B, 4H)
    h  = LSTM-scan(xg, W_hh_fwd)             # (T, B, H)
    out = h @ W_out.T + b_out                # (B, T, V=32000)

Distribution strategy:
  - Embedding lookup: host-side (pure indexed copy of inputs).
  - xg GEMM: sharded over T across the 8 cores (fp32r), AllGathered.
  - LSTM scan: replicated on all 8 cores (sequential recurrence; weights
    stationary in bf16 to exploit fast-weight-load; gates kept with the
    gate dim on partitions; per-gate-group PSUM tiles so the elementwise
    chain pipelines against the matmuls).
  - Output GEMM: vocab-sharded (4000 vocab rows per core, fp32r),
    interleaved into the scan's idle PE slots; each core writes its
    (8, 512, 4000) slice; host concatenates.
"""

import os
import numpy as np
import ml_dtypes

import concourse.bass as bass
import concourse.tile as tile
from concourse import bacc, mybir
from concourse.bass_utils import run_bass_kernel_spmd

F32 = mybir.dt.float32
F32R = mybir.dt.float32r
BF16 = mybir.dt.bfloat16
AF = mybir.ActivationFunctionType

N_CORES = 8
B, T, E, H, V = 8, 512, 512, 512, 32000
G = 4 * H                   # 2048 gate rows
NM = G // 128               # 16 gate M-tiles
NK = H // 128               # 4 contraction K-tiles
TC = T // N_CORES           # 64 timesteps per core for the xg GEMM
NS = 8                      # xg AllGather sub-chunks per core (8 steps each)
TS = TC // NS               # timesteps per AG sub-chunk
VC = V // N_CORES           # 4000 vocab rows per core
VCH = 8                     # vocab chunks in output GEMM
VN = VC // VCH              # 500 vocab per chunk
NBT = (B * T) // 128        # 32 bt-tiles in the output GEMM

_T_BUILD = int(os.environ.get("BILSTM_T_BUILD", "512"))  # dev knob: scan length

# gate m-tile group order: f(0:4) i(4:8) g(8:12) o(12:16) — f first so its
# sigmoid can start while later groups' matmuls still stream.
_PERM = np.concatenate([np.arange(H, 2 * H), np.arange(0, H),
                        np.arange(2 * H, 3 * H), np.arange(3 * H, 4 * H)])

_CACHE = {}


def _wire_ntff_hook():
    """The agent image's antenv lacks axon_hooks; synthesize it so
    run_bass_kernel_spmd(trace=True) can capture NTFF profiles."""
    import sys
    import types
    try:
        from antenv.axon_hooks import get_axon_ntff_profile_hook  # noqa: F401
        return
    except ImportError:
        pass
    try:
        import antenv
        from trn_agent_boot.trn_boot import _ntff_profile_via_ctypes
        mod = types.ModuleType("antenv.axon_hooks")
        _store = [None]
        mod.set_axon_ntff_profile_hook = lambda h: _store.__setitem__(0, h)
        mod.get_axon_ntff_profile_hook = lambda: _store[0]
        sys.modules["antenv.axon_hooks"] = mod
        antenv.axon_hooks = mod
        mod.set_axon_ntff_profile_hook(
            _ntff_profile_via_ctypes("/opt/axon/libaxon_pjrt.so"))
    except Exception:
        pass


_wire_ntff_hook()


def _build():
    if "nc" in _CACHE:
        return _CACHE["nc"]
    nc = bacc.Bacc("TRN2", target_bir_lowering=False, debug=False,
                   num_devices=N_CORES)

    # ---- DRAM I/O ----
    xt_dram = nc.dram_tensor("xt", [E, TC * B], F32R, kind="ExternalInput")
    wih_dram = nc.dram_tensor("wih", [E, G], F32R, kind="ExternalInput")
    whh_dram = nc.dram_tensor("whh", [H, G], BF16, kind="ExternalInput")
    bg_dram = nc.dram_tensor("bg", [128, NM], F32, kind="ExternalInput")
    wout_dram = nc.dram_tensor("wout", [H, VC], F32R, kind="ExternalInput")
    bout_dram = nc.dram_tensor("bout", [128, VC], F32, kind="ExternalInput")
    out_dram = nc.dram_tensor("out", [B, T, VC], F32, kind="ExternalOutput")

    xg_mine = [nc.dram_tensor(f"xg_mine{s}", [NM, 128, TS * B], F32)
               for s in range(NS)]
    xg_all = [nc.dram_tensor(f"xg_all{s}", [N_CORES, NM, 128, TS * B], F32,
                             addr_space="Shared") for s in range(NS)]

    with tile.TileContext(nc) as tc:
        with (
            tc.tile_pool(name="wbig", bufs=1) as wbig,      # W_ihT then W_outT
            tc.tile_pool(name="wsmall", bufs=1) as wsmall,  # persistent weights
            tc.tile_pool(name="state", bufs=1) as statep,   # scan state
            tc.tile_pool(name="hs", bufs=NBT) as hsp,       # scan outputs
            tc.tile_pool(name="xgst", bufs=3) as xgst,      # xg staging
            tc.tile_pool(name="xgpre", bufs=4) as xgpre,    # scan xg prefetch
            tc.tile_pool(name="gt", bufs=2) as gtp,         # gate tiles
            tc.tile_pool(name="ovec", bufs=4) as ovec,      # out staging
            tc.tile_pool(name="psg", bufs=2, space="PSUM") as psgp,   # gemm psum
            tc.tile_pool(name="psf", bufs=1, space="PSUM") as ps_f,
            tc.tile_pool(name="psi", bufs=1, space="PSUM") as ps_i,
            tc.tile_pool(name="psgg", bufs=1, space="PSUM") as ps_g,
            tc.tile_pool(name="pso", bufs=1, space="PSUM") as ps_o,
        ):
            grp_pools = [ps_f, ps_i, ps_g, ps_o]

            # ================= phase 0: weight loads =================
            wih = wbig.tile([128, NK, G], F32R, tag="wbig")
            nc.sync.dma_start(wih[:], wih_dram[:].rearrange("(k p) g -> p k g", p=128))
            whh = wsmall.tile([128, NK, G], BF16)
            nc.sync.dma_start(whh[:], whh_dram[:].rearrange("(k p) g -> p k g", p=128))
            bg = wsmall.tile([128, NM], F32)
            nc.sync.dma_start(bg[:], bg_dram[:])
            xt = wsmall.tile([128, NK, TC * B], F32R)
            nc.sync.dma_start(xt[:], xt_dram[:].rearrange("(k p) n -> p k n", p=128))

            # ================= phase 1: xg GEMM (my T-chunk) =================
            for m in range(NM):
                ps = psgp.tile([128, TC * B], F32, tag="psg", name=f"xg_ps{m}")
                for k in range(NK):
                    nc.tensor.matmul(
                        ps[:], wih[:, k, 128 * m:128 * (m + 1)], xt[:, k, :],
                        start=(k == 0), stop=(k == NK - 1))
                st = xgst.tile([128, TC * B], F32, tag="xgst", name=f"xg_st{m}")
                nc.scalar.activation(st[:], ps[:], AF.Identity,
                                     bias=bg[:, m:m + 1])
                for s in range(NS):
                    nc.sync.dma_start(
                        xg_mine[s][m], st[:, TS * B * s:TS * B * (s + 1)])

            # ================= phase 2: AllGather xg =================
            for s in range(NS):
                nc.gpsimd.collective_compute(
                    "AllGather", mybir.AluOpType.bypass,
                    ins=[xg_mine[s][:]], outs=[xg_all[s][:]],
                    replica_groups=[list(range(N_CORES))])

            # W_outT load (overlaps the scan; reuses the W_ihT slot)
            wout = wbig.tile([128, NK, VC], F32R, tag="wbig")
            nc.sync.dma_start(wout[:], wout_dram[:].rearrange("(k p) v -> p k v", p=128))
            bout = wsmall.tile([128, VC], F32)
            nc.sync.dma_start(bout[:], bout_dram[:])

            # ================= phase 3+4: LSTM scan + interleaved out-GEMM ====
            c_t = statep.tile([128, NK, B], F32)
            hbf = statep.tile([128, NK, B], BF16)
            t1 = statep.tile([128, NK, B], F32)
            t2 = statep.tile([128, NK, B], F32)
            tnc = statep.tile([128, NK, B], F32)
            nc.vector.memset(c_t[:], 0.0)
            nc.vector.memset(hbf[:].bitcast(mybir.dt.uint16), 0)

            hs = [hsp.tile([128, NK, 128], F32R, tag="hs", name=f"hs{j}")
                  for j in range(NBT)]
            for hst in hs:
                nc.vector.memset(hst[:].bitcast(F32), 0.0)

            gemm_ps = {}   # v-chunk psum tiles for the interleaved out-GEMM

            def emit_gemm_mm(j, v, k):
                if k == 0:
                    gemm_ps[(j, v)] = psgp.tile(
                        [128, VN], F32, tag="psg", name=f"gps{j}_{v}")
                nc.tensor.matmul(
                    gemm_ps[(j, v)][:], hs[j][:, k, :],
                    wout[:, k, VN * v:VN * (v + 1)],
                    start=(k == 0), stop=(k == NK - 1),
                    skip_group_check=True)

            def emit_gemm_out(j, v):
                ps = gemm_ps.pop((j, v))
                ot = ovec.tile([128, VN], F32, tag="ot", name=f"ot{j}_{v}")
                nc.vector.tensor_add(ot[:], ps[:], bout[:, VN * v:VN * (v + 1)])
                dst = out_dram[:, 16 * j:16 * (j + 1), VN * v:VN * (v + 1)]
                nc.sync.dma_start(dst.rearrange("b t v -> t b v"), ot[:])

            FUNCS = [AF.Sigmoid, AF.Sigmoid, AF.Tanh, AF.Sigmoid]
            for t in range(_T_BUILD):
                cc, ss, tl = t // TC, (t % TC) // TS, t % TS
                xg_t = xgpre.tile([128, NM, B], F32, tag="xg_t", name=f"xg_t{t}")
                nc.sync.dma_start(
                    xg_t[:],
                    xg_all[ss][cc, :, :, B * tl:B * (tl + 1)].rearrange(
                        "m p b -> p m b"))

                gtiles = []
                for grp in range(4):
                    ps = grp_pools[grp].tile([128, 4, B], F32, tag=f"ps{grp}",
                                             name=f"ps{grp}_{t}")
                    for ml in range(4):
                        m = 4 * grp + ml
                        for k in range(NK):
                            nc.tensor.matmul(
                                ps[:, ml, :],
                                whh[:, k, 128 * m:128 * (m + 1)], hbf[:, k, :],
                                start=(k == 0), stop=(k == NK - 1))
                    gt = gtp.tile([128, 4, B], F32, tag=f"g{grp}",
                                  name=f"g{grp}_{t}")
                    nc.vector.tensor_add(gt[:], ps[:], xg_t[:, 4 * grp:4 * (grp + 1), :])
                    nc.scalar.activation(gt[:], gt[:], FUNCS[grp])
                    gtiles.append(gt)

                gf, gi, gg, go = gtiles
                nc.vector.tensor_mul(t2[:], gf[:], c_t[:])
                nc.vector.tensor_mul(t1[:], gi[:], gg[:])
                nc.vector.tensor_add(c_t[:], t1[:], t2[:])
                nc.scalar.activation(tnc[:], c_t[:], AF.Tanh)
                nc.vector.tensor_mul(hbf[:], go[:], tnc[:])
                j, o = t // 16, t % 16
                nc.vector.tensor_mul(hs[j][:, :, B * o:B * (o + 1)], go[:], tnc[:])

                # interleave the previous bt-tile's output GEMM (2 MMs/step)
                jj = t // 16 - 1
                if 0 <= jj < NBT:
                    idx = t % 16
                    for pair in (2 * idx, 2 * idx + 1):
                        v, k = divmod(pair, NK)
                        emit_gemm_mm(jj, v, k)
                        if k == NK - 1:
                            emit_gemm_out(jj, v)

            # tail: last bt-tile (and any skipped when _T_BUILD < T)
            done_j = max(0, _T_BUILD // 16 - 1)
            for j in range(done_j, NBT):
                for v in range(VCH):
                    for k in range(NK):
                        emit_gemm_mm(j, v, k)
                    emit_gemm_out(j, v)

    nc.compile()
    _CACHE["nc"] = nc
    return nc


def kernel(**inputs) -> np.ndarray:
    inp = np.asarray(inputs["input"])
    emb = np.asarray(inputs["emb"], dtype=np.float32)
    W_ih = np.asarray(inputs["W_ih_fwd"], dtype=np.float32)
    b_ih = np.asarray(inputs["b_ih_fwd"], dtype=np.float32)
    W_hh = np.asarray(inputs["W_hh_fwd"], dtype=np.float32)
    b_hh = np.asarray(inputs["b_hh_fwd"], dtype=np.float32)
    W_out = np.asarray(inputs["W_out"], dtype=np.float32)
    b_out = np.asarray(inputs["b_out"], dtype=np.float32)

    nc = _build()

    # host-side input prep
    x = emb[inp]                                   # (B, T, E)
    wihT = np.ascontiguousarray(W_ih[_PERM].T)     # (E, G) permuted gate order
    whhT = np.ascontiguousarray(W_hh[_PERM].T).astype(ml_dtypes.bfloat16)
    bgv = (b_ih + b_hh)[_PERM].reshape(NM, 128).T.copy()  # (128, NM)

    in_maps = []
    for c in range(N_CORES):
        xc = x[:, TC * c:TC * (c + 1), :]          # (B, TC, E)
        xt = np.ascontiguousarray(xc.transpose(2, 1, 0).reshape(E, TC * B))
        wo = np.ascontiguousarray(W_out[VC * c:VC * (c + 1)].T)  # (H, VC)
        bo = np.tile(b_out[VC * c:VC * (c + 1)][None, :], (128, 1))
        in_maps.append({
            "xt": xt, "wih": wihT, "whh": whhT, "bg": bgv,
            "wout": wo, "bout": np.ascontiguousarray(bo),
        })

    res = run_bass_kernel_spmd(
        nc, in_maps, core_ids=list(range(N_CORES)),
        trace=bool(int(os.environ.get("BILSTM_TRACE", "0"))))
    _CACHE["last_res"] = res
    out = np.concatenate([res.results[c]["out"] for c in range(N_CORES)], axis=2)
    return out.astype(np.float32)
